# revision 5
# baseline (speedup 1.0000x reference)
"""Trainium2 Bass kernel for nn_ConditionalSplineSQ2D.

Math:
  out[b] = sum_{g,h,c} coeffs[g,h,c] * p[b,g,h,ii_c] * p[b,g,h,jj_c]
         = sum_{cells} p_cell^T S_cell p_cell            (S_cell symmetric 8x8)

Two equivalent per-group forms, split across engines to balance the
PSUM-drain work (the kernel's second bottleneck after HBM traffic):
  EIGEN  (ACT):  T = V^T p (PE), Q = T*T (ScalarE square), out += lam^T Q (PE)
  DIRECT (DVE):  T = S^T p (PE), Z = T .* p (VectorE fused mult), out += 1^T Z (PE)

Groups are processed in waves of 2: EE waves drain with one fused ACT
square, DD waves with one fused DVE multiply, so each engine pays its
per-instruction overhead once per kilobyte-scale op.

mm1 uses the 16x 32x32 PE sub-array tiling: each group's block-diag
stationary is 4 independent 32x32 tiles (4 cells each).  Eigen groups
rotate tiles to column band (r+g)%4 so concurrent groups occupy
disjoint sub-arrays; direct groups stay on the diagonal (c=r) so the
PSUM partition layout matches pt's layout for the elementwise multiply.

The entire per-core input (~8 MB fp16) streams into resident SBUF tiles
through two parallel HWDGE rings (pt on Sync, weights+lam on ScalarE)
with the first chunks kept small so mm1 starts by ~3 us.

Sharding: pure data parallel over batch (512 per core x 8 cores); the
4 partial accumulator rows per core are summed on host.
"""

import numpy as np

B, G, P = 4096, 31, 8
NCORES = 8
NB = B // NCORES  # 512 batches per core
CELLS = G * G  # 961
GROUP_CELLS = 16
NGROUPS = -(-CELLS // GROUP_CELLS)  # 61
CELLS_PAD = NGROUPS * GROUP_CELLS  # 976
PARTS = 128
WAVE = 2  # groups per wave (one PSUM bank per group)
NWAVES = -(-NGROUPS // WAVE)  # 31 (last wave has 1 group)

# pt DMA chunks (in groups).  Boundaries must not split a DD wave
# (i.e. never fall on g % 4 == 3) so fused DVE drains stay contiguous.
PT_CHUNKS = [2, 4, 8, 10, 12, 12, 13]
assert sum(PT_CHUNKS) == NGROUPS
assert all(b % 4 != 3 for b in np.cumsum(PT_CHUNKS[:-1]))
WL_CHUNKS = [12, NGROUPS - 12]  # weight+lam chunks (in groups)
WL_LAM = NGROUPS  # lam columns at the front of the wl tensor


def _group_form(g):
    """'E' (eigen, ACT square) or 'D' (direct, DVE multiply)."""
    return "E" if ((g // 2) % 2 == 0 or g == NGROUPS - 1) else "D"


def _col_band(g, r):
    """PE column band for tile r of group g (rotated for E, diagonal for D)."""
    return (r + g) % 4 if _group_form(g) == "E" else r


_nc_cache = {}


def _build_nc():
    import concourse.mybir as mybir
    import concourse.tile as tile
    from concourse import bacc

    nc = bacc.Bacc()
    pt_d = nc.dram_tensor(
        "pt", [PARTS, NGROUPS * NB], mybir.dt.float16, kind="ExternalInput"
    )
    wl_d = nc.dram_tensor(
        "wl", [PARTS, WL_LAM + NGROUPS * 32], mybir.dt.float16, kind="ExternalInput"
    )
    out_d = nc.dram_tensor("out", [4, NB], mybir.dt.float32, kind="ExternalOutput")

    with tile.TileContext(nc) as tc:
        with (
            tc.tile_pool(name="const", bufs=1) as cpool,
            tc.tile_pool(name="wlp", bufs=len(WL_CHUNKS)) as wpool,
            tc.tile_pool(name="ptp", bufs=len(PT_CHUNKS)) as ppool,
            tc.tile_pool(name="qp", bufs=3) as qpool,
            tc.tile_pool(name="psp", bufs=3, space="PSUM") as pspool,
            tc.tile_pool(name="accp", bufs=1, space="PSUM") as apool,
        ):
            acc = apool.tile([PARTS, NB], mybir.dt.float32)
            out_sb = cpool.tile([PARTS, NB], mybir.dt.float32)

            # ---- input DMAs: pt on Sync ring, weights+lam on ScalarE ring ----
            wtiles = []  # (tile, first_group, ngroups)
            g0 = 0
            for ci, ch in enumerate(WL_CHUNKS):
                ncols = (WL_LAM if ci == 0 else 0) + ch * 32
                wt = wpool.tile(
                    [PARTS, WL_LAM + max(WL_CHUNKS) * 32], mybir.dt.float16, tag="wl"
                )
                src0 = (WL_LAM if ci > 0 else 0) + g0 * 32
                nc.scalar.dma_start(
                    out=wt[:, :ncols], in_=wl_d[:, src0 : src0 + ncols]
                )
                wtiles.append((wt, g0, ch))
                g0 += ch
            lam_sb = wtiles[0][0]  # cols [0, WL_LAM) of chunk 0

            ptiles = []
            group_pt = {}  # group -> (tile_idx, offset_in_chunk)
            g0 = 0
            for ci, ch in enumerate(PT_CHUNKS):
                pt = ppool.tile([PARTS, max(PT_CHUNKS) * NB], mybir.dt.float16,
                                tag="pt")
                nc.sync.dma_start(
                    out=pt[:, : ch * NB],
                    in_=pt_d[:, g0 * NB : (g0 + ch) * NB],
                )
                ptiles.append(pt)
                for k in range(ch):
                    group_pt[g0 + k] = (ci, k)
                g0 += ch

            def wt_slice(g, r):
                for wt, wg0, wch in wtiles:
                    if wg0 <= g < wg0 + wch:
                        c0 = (WL_LAM if wg0 == 0 else 0) + (g - wg0) * 32
                        return wt[32 * r : 32 * r + 32, c0 : c0 + 32]
                raise AssertionError(g)

            # ---- pipeline over waves ----
            q_slices = {}  # group -> (q_tile, slot)
            n_rounds = -(-NGROUPS // 4)
            rounds_emitted = 0
            last_round_of_pos = {}
            for r in range(n_rounds):
                for j in range(4):
                    if r * 4 + j < NGROUPS:
                        last_round_of_pos[j] = r

            def evacuate_band(j):
                # acc row 32j -> out_sb row 32j (DMA remaps partitions)
                if j % 2:
                    nc.scalar.copy(
                        out_sb[32 * j : 32 * j + 1, :], acc[32 * j : 32 * j + 1, :]
                    )
                else:
                    nc.vector.tensor_copy(
                        out_sb[32 * j : 32 * j + 1, :], acc[32 * j : 32 * j + 1, :]
                    )

            def emit_mm2_rounds(limit_group):
                nonlocal rounds_emitted
                while rounds_emitted < n_rounds:
                    r = rounds_emitted
                    hi = min(r * 4 + 4, NGROUPS)
                    if hi > limit_group:
                        return
                    for j in range(4):
                        g = r * 4 + j
                        if g >= NGROUPS:
                            break
                        qt, slot = q_slices.pop(g)
                        nc.tensor.matmul(
                            acc[32 * j : 32 * j + 1, :],
                            lam_sb[:, g : g + 1],
                            qt[:, slot * NB : (slot + 1) * NB],
                            start=(r == 0),
                            stop=(r == last_round_of_pos[j]),
                            tile_position=(0, 32 * j),
                        )
                        if r == last_round_of_pos[j]:
                            evacuate_band(j)
                    rounds_emitted += 1

            for w in range(NWAVES):
                wg0 = w * WAVE
                ng = min(WAVE, NGROUPS - wg0)
                psT = pspool.tile([PARTS, WAVE * NB], mybir.dt.float32, tag="psT")
                for k in range(ng):
                    g = wg0 + k
                    ci, off = group_pt[g]
                    for r in range(4):
                        c = _col_band(g, r)
                        nc.tensor.matmul(
                            psT[32 * c : 32 * c + 32, k * NB : (k + 1) * NB],
                            wt_slice(g, r),
                            ptiles[ci][32 * r : 32 * r + 32,
                                       off * NB : (off + 1) * NB],
                            start=True,
                            stop=True,
                            tile_position=(32 * r, 32 * c),
                        )
                # drain PSUM: one fused op per wave
                q = qpool.tile([PARTS, WAVE * NB], mybir.dt.float16, tag="q")
                if _group_form(wg0) == "E":
                    nc.scalar.square(q[:, : ng * NB], psT[:, : ng * NB])
                else:
                    k = 0
                    while k < ng:  # split only at pt-chunk boundaries
                        ci, off = group_pt[wg0 + k]
                        k1 = k + 1
                        while k1 < ng and group_pt[wg0 + k1] == (ci, off + k1 - k):
                            k1 += 1
                        nc.vector.tensor_mul(
                            q[:, k * NB : k1 * NB],
                            psT[:, k * NB : k1 * NB],
                            ptiles[ci][:, off * NB : (off + k1 - k) * NB],
                        )
                        k = k1
                for k in range(ng):
                    q_slices[wg0 + k] = (q, k)
                emit_mm2_rounds(wg0)
            emit_mm2_rounds(NGROUPS)

            # single strided DMA for the 4 partial-sum rows
            nc.sync.dma_start(out=out_d[:, :], in_=out_sb[0:97:32, :])
    if not nc.is_finalized():
        nc.finalize()
    return nc


def _get_nc():
    if "nc" not in _nc_cache:
        _nc_cache["nc"] = _build_nc()
    return _nc_cache["nc"]


def _host_prep_weights(integral_coeffs):
    """coeffs [G,G,C] -> wl [128, NGROUPS + NGROUPS*32] fp16 (lam | wblk).

    Per group g, tile r: wblk[32r + 8q + i, 32g + 8q + k] holds, for cell
    ct = 16g + 4r + q, either V_ct[i,k] (eigen groups) or S_ct[i,k]
    (direct groups).  lam[32c + 8q + k, g] holds the matching psum-
    partition weight for mm2: lam_ct[k] with r=(c-g)%4 for eigen groups,
    1.0 for direct groups.
    """
    ii, jj = np.triu_indices(P)
    wq = integral_coeffs.reshape(CELLS, len(ii)).astype(np.float64)
    S = np.zeros((CELLS, P, P), np.float64)
    np.add.at(S, (slice(None), ii, jj), 0.5 * wq)
    np.add.at(S, (slice(None), jj, ii), 0.5 * wq)
    lam, V = np.linalg.eigh(S)

    S_p = np.zeros((CELLS_PAD, P, P))
    S_p[:CELLS] = S
    lam_p = np.zeros((CELLS_PAD, P))
    lam_p[:CELLS] = lam
    V_p = np.zeros((CELLS_PAD, P, P))
    V_p[:CELLS] = V

    wblk = np.zeros((PARTS, NGROUPS * 32), np.float32)
    lamt = np.zeros((PARTS, NGROUPS), np.float32)
    for g in range(NGROUPS):
        eigen = _group_form(g) == "E"
        M = V_p if eigen else S_p
        for r in range(4):
            for q in range(4):
                ct = 16 * g + 4 * r + q
                wblk[32 * r + 8 * q : 32 * r + 8 * q + 8,
                     32 * g + 8 * q : 32 * g + 8 * q + 8] = M[ct]
        if eigen:
            for c in range(4):
                r = (c - g) % 4
                for q in range(4):
                    ct = 16 * g + 4 * r + q
                    lamt[32 * c + 8 * q : 32 * c + 8 * q + 8, g] = lam_p[ct]
        else:
            lamt[:, g] = 1.0
    wl = np.concatenate([lamt, wblk], axis=1).astype(np.float16)
    return np.ascontiguousarray(wl)


def _host_prep_param(param_tensor):
    """param [B,G,G,P] f32 -> list of per-core [128, NGROUPS*NB] fp16 arrays."""
    flat = param_tensor.reshape(B, CELLS * P)
    out = []
    for c in range(NCORES):
        shard = flat[c * NB : (c + 1) * NB]
        pad = np.zeros((NB, CELLS_PAD * P), np.float32)
        pad[:, : CELLS * P] = shard
        # (b, g, p) -> (p, g, b)
        pt = (
            pad.reshape(NB, NGROUPS, PARTS)
            .transpose(2, 1, 0)
            .reshape(PARTS, NGROUPS * NB)
            .astype(np.float16)
        )
        out.append(np.ascontiguousarray(pt))
    return out


def _run(param_tensor, integral_coeffs, trace=False, **run_kwargs):
    from concourse.bass_utils import run_bass_kernel_spmd

    nc = _get_nc()
    wl = _host_prep_weights(np.asarray(integral_coeffs, np.float32))
    pts = _host_prep_param(np.asarray(param_tensor, np.float32))
    in_maps = [{"pt": pts[c], "wl": wl} for c in range(NCORES)]
    res = run_bass_kernel_spmd(
        nc, in_maps, core_ids=list(range(NCORES)), trace=trace, **run_kwargs
    )
    out = np.concatenate(
        [res.results[c]["out"].sum(axis=0).reshape(NB) for c in range(NCORES)]
    ).astype(np.float32)
    return out, res


def kernel(param_tensor, integral_coeffs):
    out, _ = _run(param_tensor, integral_coeffs)
    return out


# revision 8
# speedup vs baseline: 1.2618x; 1.2618x over previous
"""Trainium2 Bass kernel for nn_ConditionalSplineSQ2D.

Math:
  out[b] = sum_{g,h,c} coeffs[g,h,c] * p[b,g,h,ii_c] * p[b,g,h,jj_c]
         = sum_{cells} p_cell^T S_cell p_cell            (S_cell symmetric 8x8)

Two equivalent per-group forms, split across engines to balance the
PSUM-drain work (the kernel's second bottleneck after HBM traffic):
  EIGEN  (ACT):  T = V^T p (PE), Q = T*T (ScalarE square), out += lam^T Q (PE)
  DIRECT (DVE):  T = S^T p (PE), Z = T .* p (VectorE fused mult), out += 1^T Z (PE)

Groups are processed in waves of 3 (EED/EDD patterns): each wave's
eigen span drains with one fused ACT square and its direct span with
one fused DVE multiply, so each engine pays its per-instruction
overhead once per multi-KB op.

mm1 uses the 16x 32x32 PE sub-array tiling: each group's block-diag
stationary is 4 independent 32x32 tiles (4 cells each).  Direct groups
must sit on the diagonal sub-arrays (c=r) so the PSUM partition layout
matches pt's layout for the elementwise multiply; eigen groups rotate
to column band (r+sigma)%4 with sigma in 1..3, keeping them off the
diagonals that direct groups and mm2 column bands already load.

The entire per-core input (~8 MB fp16) streams into resident SBUF tiles
through two parallel HWDGE rings (pt on Sync, weights+lam on ScalarE)
with the first chunks kept small so mm1 starts by ~3 us.

Sharding: pure data parallel over batch (512 per core x 8 cores); the
4 partial accumulator rows per core are summed on host.
"""

import numpy as np

B, G, P = 4096, 31, 8
NCORES = 8
NB = B // NCORES  # 512 batches per core
CELLS = G * G  # 961
GROUP_CELLS = 16
NGROUPS = -(-CELLS // GROUP_CELLS)  # 61
CELLS_PAD = NGROUPS * GROUP_CELLS  # 976
PARTS = 128
WAVE = 3  # groups per wave (one PSUM bank per group, 3-bank wave tiles)
NWAVES = -(-NGROUPS // WAVE)  # 21 (last wave has 1 group)

PT_CHUNKS = [3, 6, 10, 10, 12, 10, 10]  # pt DMA chunks (in groups)
assert sum(PT_CHUNKS) == NGROUPS
WL_CHUNKS = [12, NGROUPS - 12]  # weight+lam chunks (in groups)
WL_LAM = NGROUPS  # lam columns at the front of the wl tensor


def _wave_pattern(w):
    """Group forms for wave w, E(igen/ACT) first then D(irect/DVE).

    14 EED waves + 7 EDD waves + final E balance ACT ~18us vs DVE ~17us."""
    if w == NWAVES - 1 and NGROUPS % WAVE:
        return "E" * (NGROUPS % WAVE)
    return "EDD" if w % 3 == 2 else "EED"


def _group_form(g):
    w, k = divmod(g, WAVE)
    return _wave_pattern(w)[k]


def _sigma(g):
    """Column-band rotation for eigen groups; 1..3 keeps them off the
    diagonal sub-arrays, which direct groups and mm2 already load."""
    return 1 + (g % 3)


def _col_band(g, r):
    """PE column band for tile r of group g (rotated for E, diagonal for D)."""
    return (r + _sigma(g)) % 4 if _group_form(g) == "E" else r


_nc_cache = {}


def _build_nc():
    import concourse.mybir as mybir
    import concourse.tile as tile
    from concourse import bacc

    nc = bacc.Bacc()
    pt_d = nc.dram_tensor(
        "pt", [PARTS, NGROUPS * NB], mybir.dt.float16, kind="ExternalInput"
    )
    wl_d = nc.dram_tensor(
        "wl", [PARTS, WL_LAM + NGROUPS * 32], mybir.dt.float16, kind="ExternalInput"
    )
    out_d = nc.dram_tensor("out", [4, NB], mybir.dt.float32, kind="ExternalOutput")

    with tile.TileContext(nc) as tc:
        with (
            tc.tile_pool(name="const", bufs=1) as cpool,
            tc.tile_pool(name="wlp", bufs=len(WL_CHUNKS)) as wpool,
            tc.tile_pool(name="ptp", bufs=len(PT_CHUNKS)) as ppool,
            tc.tile_pool(name="qp", bufs=3) as qpool,
            tc.tile_pool(name="psp", bufs=2, space="PSUM") as pspool,
            tc.tile_pool(name="accp", bufs=1, space="PSUM") as apool,
        ):
            acc = apool.tile([PARTS, NB], mybir.dt.float32)
            out_sb = cpool.tile([PARTS, NB], mybir.dt.float32)

            # ---- input DMAs: pt on Sync ring, weights+lam on ScalarE ring ----
            wtiles = []  # (tile, first_group, ngroups)
            g0 = 0
            for ci, ch in enumerate(WL_CHUNKS):
                ncols = (WL_LAM if ci == 0 else 0) + ch * 32
                wt = wpool.tile(
                    [PARTS, WL_LAM + max(WL_CHUNKS) * 32], mybir.dt.float16, tag="wl"
                )
                src0 = (WL_LAM if ci > 0 else 0) + g0 * 32
                nc.scalar.dma_start(
                    out=wt[:, :ncols], in_=wl_d[:, src0 : src0 + ncols]
                )
                wtiles.append((wt, g0, ch))
                g0 += ch
            lam_sb = wtiles[0][0]  # cols [0, WL_LAM) of chunk 0

            ptiles = []
            group_pt = {}  # group -> (tile_idx, offset_in_chunk)
            g0 = 0
            for ci, ch in enumerate(PT_CHUNKS):
                pt = ppool.tile([PARTS, max(PT_CHUNKS) * NB], mybir.dt.float16,
                                tag="pt")
                nc.sync.dma_start(
                    out=pt[:, : ch * NB],
                    in_=pt_d[:, g0 * NB : (g0 + ch) * NB],
                )
                ptiles.append(pt)
                for k in range(ch):
                    group_pt[g0 + k] = (ci, k)
                g0 += ch

            def wt_slice(g, r):
                for wt, wg0, wch in wtiles:
                    if wg0 <= g < wg0 + wch:
                        c0 = (WL_LAM if wg0 == 0 else 0) + (g - wg0) * 32
                        return wt[32 * r : 32 * r + 32, c0 : c0 + 32]
                raise AssertionError(g)

            # ---- pipeline over waves ----
            q_slices = {}  # group -> (q_tile, slot)
            n_rounds = -(-NGROUPS // 4)
            rounds_emitted = 0
            last_round_of_pos = {}
            for r in range(n_rounds):
                for j in range(4):
                    if r * 4 + j < NGROUPS:
                        last_round_of_pos[j] = r

            def evacuate_band(j):
                # acc row 32j -> out_sb row 32j (DMA remaps partitions)
                if j % 2:
                    nc.scalar.copy(
                        out_sb[32 * j : 32 * j + 1, :], acc[32 * j : 32 * j + 1, :]
                    )
                else:
                    nc.vector.tensor_copy(
                        out_sb[32 * j : 32 * j + 1, :], acc[32 * j : 32 * j + 1, :]
                    )

            def emit_mm2_rounds(limit_group):
                nonlocal rounds_emitted
                while rounds_emitted < n_rounds:
                    r = rounds_emitted
                    hi = min(r * 4 + 4, NGROUPS)
                    if hi > limit_group:
                        return
                    for j in range(4):
                        g = r * 4 + j
                        if g >= NGROUPS:
                            break
                        qt, slot = q_slices.pop(g)
                        nc.tensor.matmul(
                            acc[32 * j : 32 * j + 1, :],
                            lam_sb[:, g : g + 1],
                            qt[:, slot * NB : (slot + 1) * NB],
                            start=(r == 0),
                            stop=(r == last_round_of_pos[j]),
                            tile_position=(0, 32 * j),
                        )
                        if r == last_round_of_pos[j]:
                            evacuate_band(j)
                    rounds_emitted += 1

            for w in range(NWAVES):
                wg0 = w * WAVE
                ng = min(WAVE, NGROUPS - wg0)
                psT = pspool.tile([PARTS, WAVE * NB], mybir.dt.float32, tag="psT")
                for k in range(ng):
                    g = wg0 + k
                    ci, off = group_pt[g]
                    for r in range(4):
                        c = _col_band(g, r)
                        nc.tensor.matmul(
                            psT[32 * c : 32 * c + 32, k * NB : (k + 1) * NB],
                            wt_slice(g, r),
                            ptiles[ci][32 * r : 32 * r + 32,
                                       off * NB : (off + 1) * NB],
                            start=True,
                            stop=True,
                            tile_position=(32 * r, 32 * c),
                        )
                # drain PSUM: ACT squares the eigen span, DVE fuses the
                # direct span with pt (one op, no copy)
                q = qpool.tile([PARTS, WAVE * NB], mybir.dt.float16, tag="q")
                n_e = _wave_pattern(w).count("E")
                if n_e:
                    nc.scalar.square(q[:, : n_e * NB], psT[:, : n_e * NB])
                k = n_e
                while k < ng:  # direct span, split at pt-chunk boundaries
                        ci, off = group_pt[wg0 + k]
                        k1 = k + 1
                        while k1 < ng and group_pt[wg0 + k1] == (ci, off + k1 - k):
                            k1 += 1
                        nc.vector.tensor_mul(
                            q[:, k * NB : k1 * NB],
                            psT[:, k * NB : k1 * NB],
                            ptiles[ci][:, off * NB : (off + k1 - k) * NB],
                        )
                        k = k1
                for k in range(ng):
                    q_slices[wg0 + k] = (q, k)
                emit_mm2_rounds(wg0)
            emit_mm2_rounds(NGROUPS)

            # single strided DMA for the 4 partial-sum rows
            nc.sync.dma_start(out=out_d[:, :], in_=out_sb[0:97:32, :])
    if not nc.is_finalized():
        nc.finalize()
    return nc


def _get_nc():
    if "nc" not in _nc_cache:
        _nc_cache["nc"] = _build_nc()
    return _nc_cache["nc"]


def _host_prep_weights(integral_coeffs):
    """coeffs [G,G,C] -> wl [128, NGROUPS + NGROUPS*32] fp16 (lam | wblk).

    Per group g, tile r: wblk[32r + 8q + i, 32g + 8q + k] holds, for cell
    ct = 16g + 4r + q, either V_ct[i,k] (eigen groups) or S_ct[i,k]
    (direct groups).  lam[32c + 8q + k, g] holds the matching psum-
    partition weight for mm2: lam_ct[k] with r=(c-g)%4 for eigen groups,
    1.0 for direct groups.
    """
    ii, jj = np.triu_indices(P)
    wq = integral_coeffs.reshape(CELLS, len(ii)).astype(np.float64)
    S = np.zeros((CELLS, P, P), np.float64)
    np.add.at(S, (slice(None), ii, jj), 0.5 * wq)
    np.add.at(S, (slice(None), jj, ii), 0.5 * wq)
    lam, V = np.linalg.eigh(S)

    S_p = np.zeros((CELLS_PAD, P, P))
    S_p[:CELLS] = S
    lam_p = np.zeros((CELLS_PAD, P))
    lam_p[:CELLS] = lam
    V_p = np.zeros((CELLS_PAD, P, P))
    V_p[:CELLS] = V

    wblk = np.zeros((PARTS, NGROUPS * 32), np.float32)
    lamt = np.zeros((PARTS, NGROUPS), np.float32)
    for g in range(NGROUPS):
        eigen = _group_form(g) == "E"
        M = V_p if eigen else S_p
        for r in range(4):
            for q in range(4):
                ct = 16 * g + 4 * r + q
                wblk[32 * r + 8 * q : 32 * r + 8 * q + 8,
                     32 * g + 8 * q : 32 * g + 8 * q + 8] = M[ct]
        if eigen:
            for c in range(4):
                r = (c - _sigma(g)) % 4
                for q in range(4):
                    ct = 16 * g + 4 * r + q
                    lamt[32 * c + 8 * q : 32 * c + 8 * q + 8, g] = lam_p[ct]
        else:
            lamt[:, g] = 1.0
    wl = np.concatenate([lamt, wblk], axis=1).astype(np.float16)
    return np.ascontiguousarray(wl)


def _host_prep_param(param_tensor):
    """param [B,G,G,P] f32 -> list of per-core [128, NGROUPS*NB] fp16 arrays."""
    flat = param_tensor.reshape(B, CELLS * P)
    out = []
    for c in range(NCORES):
        shard = flat[c * NB : (c + 1) * NB]
        pad = np.zeros((NB, CELLS_PAD * P), np.float32)
        pad[:, : CELLS * P] = shard
        # (b, g, p) -> (p, g, b)
        pt = (
            pad.reshape(NB, NGROUPS, PARTS)
            .transpose(2, 1, 0)
            .reshape(PARTS, NGROUPS * NB)
            .astype(np.float16)
        )
        out.append(np.ascontiguousarray(pt))
    return out


def _run(param_tensor, integral_coeffs, trace=False, **run_kwargs):
    from concourse.bass_utils import run_bass_kernel_spmd

    nc = _get_nc()
    wl = _host_prep_weights(np.asarray(integral_coeffs, np.float32))
    pts = _host_prep_param(np.asarray(param_tensor, np.float32))
    in_maps = [{"pt": pts[c], "wl": wl} for c in range(NCORES)]
    res = run_bass_kernel_spmd(
        nc, in_maps, core_ids=list(range(NCORES)), trace=trace, **run_kwargs
    )
    out = np.concatenate(
        [res.results[c]["out"].sum(axis=0).reshape(NB) for c in range(NCORES)]
    ).astype(np.float32)
    return out, res


def kernel(param_tensor, integral_coeffs):
    out, _ = _run(param_tensor, integral_coeffs)
    return out
